# revision 3
# baseline (speedup 1.0000x reference)
"""Additive (Bahdanau) attention on 8 TRN2 NeuronCores, pure data parallel.

reference:
    h = tanh(q @ Wq + c @ Wc)         # [B,L,D]
    score = squeeze(h @ Ws)           # [B,L]
    score = where(mask==1, score, -1e25)
    weight = softmax(score)[..., None]
    z = sum(weight * c, -2)           # [B,D]
    returns (weight, z)

Sharding: batch B=32 split 4-per-core across 8 cores; weights replicated.

Device layout strategy (per core, batch shard size 4):
  - context arrives twice from host as bf16: natural [4,L,D] (for the z
    matmuls, contraction over L on partitions) and pre-transposed [4,D,L]
    (for the c@Wc matmuls, contraction over D on partitions).  This avoids
    any on-chip fp32 transpose (xbar DMA transpose is 16-bit only).
  - cW^T tiles accumulate in PSUM; ScalarE applies tanh with the per-
    partition bias qW^T fused; a second PE pass contracts with Ws into a
    batched [4, L] score in PSUM (per-batch column trick in the stationary).
  - masked softmax on the [4, L] rows (DVE+ACT, fused accum for the sum).
  - softmax weights are transposed via one 16-bit xbar DMA transpose and
    used as [128,1] stationaries for the z accumulation over L tiles.
"""
import sys

sys.path.insert(0, "/opt/trn_rl_repo")

import numpy as np
import ml_dtypes
from contextlib import ExitStack

import concourse.bass as bass
import concourse.tile as tile
from concourse import bacc, mybir
from concourse.bass_utils import run_bass_kernel_spmd

F32 = mybir.dt.float32
BF16 = mybir.dt.bfloat16
I32 = mybir.dt.int32
BF = ml_dtypes.bfloat16
AF = mybir.ActivationFunctionType
ALU = mybir.AluOpType

B, L, D = 32, 4096, 256
NCORES = 8
BS = B // NCORES          # 4 batches per core
LC = 1024                 # L-chunk for the matmul/tanh pipeline
NLC = L // LC             # 4
NLT = L // 128            # 32 l-tiles of 128
MASK_FILL = -1e25

_CACHE = {}


def build():
    nc = bacc.Bacc()
    ctx_p = nc.declare_dram_parameter("ctx", [BS, L, D], BF16, isOutput=False)
    ctxT_p = nc.declare_dram_parameter("ctxT", [BS, D, L], BF16, isOutput=False)
    qT_p = nc.declare_dram_parameter("qT", [D, BS], F32, isOutput=False)
    mask_p = nc.declare_dram_parameter("mask", [BS, L], I32, isOutput=False)
    wq_p = nc.declare_dram_parameter("Wq", [D, D], F32, isOutput=False)
    wc_p = nc.declare_dram_parameter("Wc", [D, D], F32, isOutput=False)
    ws_p = nc.declare_dram_parameter("Ws2", [128, 2], F32, isOutput=False)
    w_out = nc.declare_dram_parameter("w_out", [BS, L], F32, isOutput=True)
    z_out = nc.declare_dram_parameter("z_out", [BS, D], F32, isOutput=True)

    with ExitStack() as ctx:
        tc = ctx.enter_context(tile.TileContext(nc))
        big = ctx.enter_context(tc.tile_pool(name="big", bufs=1))
        hp = ctx.enter_context(tc.tile_pool(name="hp", bufs=3))
        mp = ctx.enter_context(tc.tile_pool(name="mp", bufs=2))
        php = ctx.enter_context(tc.tile_pool(name="php", bufs=2, space="PSUM"))
        psc = ctx.enter_context(tc.tile_pool(name="psc", bufs=2, space="PSUM"))

        # ---------- resident loads ----------
        cT = []
        nat = []
        for b in range(BS):
            t = big.tile([128, 2, L], BF16, tag=f"cT{b}")
            nc.sync.dma_start(out=t, in_=ctxT_p[b].rearrange("(h p) l -> p h l", p=128))
            cT.append(t)
            t = big.tile([128, NLT, D], BF16, tag=f"nat{b}")
            nc.sync.dma_start(out=t, in_=ctx_p[b].rearrange("(lt p) d -> p lt d", p=128))
            nat.append(t)

        qT_sb = big.tile([128, 2, BS], F32, tag="qT")
        nc.sync.dma_start(out=qT_sb, in_=qT_p[:].rearrange("(h p) b -> p h b", p=128))
        wq_sb = big.tile([128, 2, D], F32, tag="wq")
        nc.sync.dma_start(out=wq_sb, in_=wq_p[:].rearrange("(h p) n -> p h n", p=128))
        wc_sb = big.tile([128, 2, D], F32, tag="wc")
        nc.sync.dma_start(out=wc_sb, in_=wc_p[:].rearrange("(h p) n -> p h n", p=128))
        ws_sb = big.tile([128, 2], F32, tag="ws")
        nc.sync.dma_start(out=ws_sb, in_=ws_p[:])

        wc_bf = big.tile([128, 2, D], BF16, tag="wcbf")
        nc.vector.tensor_copy(wc_bf, wc_sb)
        ws_bf = big.tile([128, 2], BF16, tag="wsbf")
        nc.vector.tensor_copy(ws_bf, ws_sb)

        # stationary for the batched score matmul: col b = Ws half, rest 0
        wscol = big.tile([128, 2, BS, BS], BF16, tag="wscol")
        nc.vector.memset(wscol, 0.0)
        for mh in range(2):
            for b in range(BS):
                nc.vector.tensor_copy(wscol[:, mh, b, b : b + 1], ws_bf[:, mh : mh + 1])

        negbig = big.tile([BS, 1], F32, tag="negbig")
        nc.vector.memset(negbig, MASK_FILL)

        # ---------- qW^T = (q @ Wq)^T as per-partition bias [128, mh, b] ----------
        qwt = big.tile([128, 2, BS], F32, tag="qwt")
        for mh in range(2):
            pq = php.tile([128, BS], F32, tag="ph")
            for kh in range(2):
                nc.tensor.matmul(
                    pq,
                    wq_sb[:, kh, mh * 128 : (mh + 1) * 128],
                    qT_sb[:, kh, :],
                    start=(kh == 0),
                    stop=(kh == 1),
                )
            nc.vector.tensor_copy(qwt[:, mh, :], pq)

        # ---------- main loop: cW^T -> tanh -> score ----------
        score_sb = big.tile([BS, L], F32, tag="score")
        for lc in range(NLC):
            ls = lc * LC
            sc_ps = psc.tile([BS, LC], F32, tag="sc")
            first = True
            for b in range(BS):
                for mh in range(2):
                    ph = php.tile([128, LC], F32, tag="ph")
                    for kh in range(2):
                        for nh in range(LC // 512):
                            nc.tensor.matmul(
                                ph[:, nh * 512 : (nh + 1) * 512],
                                wc_bf[:, kh, mh * 128 : (mh + 1) * 128],
                                cT[b][:, kh, ls + nh * 512 : ls + (nh + 1) * 512],
                                start=(kh == 0),
                                stop=(kh == 1),
                            )
                    hb = hp.tile([128, LC], BF16, tag="h")
                    nc.scalar.activation(hb, ph, AF.Tanh, bias=qwt[:, mh, b : b + 1])
                    for nh in range(LC // 512):
                        nc.tensor.matmul(
                            sc_ps[:, nh * 512 : (nh + 1) * 512],
                            wscol[:, mh, b, :],
                            hb[:, nh * 512 : (nh + 1) * 512],
                            start=first,
                            stop=(b == BS - 1 and mh == 1),
                            skip_group_check=True,
                        )
                    first = False
            # apply additive mask while copying PSUM -> SBUF
            mstage = mp.tile([BS, LC], I32, tag="mstage")
            nc.sync.dma_start(out=mstage, in_=mask_p[:, ls : ls + LC])
            mt = mp.tile([BS, LC], F32, tag="mt")
            nc.scalar.activation(mt, mstage, AF.Identity, bias=negbig, scale=1e25)
            nc.vector.tensor_add(score_sb[:, ls : ls + LC], sc_ps, mt)

        # ---------- masked softmax over L, batched on partitions 0..3 ----------
        negmax = big.tile([BS, 1], F32, tag="negmax")
        nc.vector.tensor_reduce(
            negmax, score_sb, axis=mybir.AxisListType.X, op=ALU.max, negate=True
        )
        sum_e = big.tile([BS, 1], F32, tag="sume")
        # in-place: score_sb becomes e = exp(score - max); sum fused
        nc.scalar.activation(score_sb, score_sb, AF.Exp, bias=negmax, accum_out=sum_e)
        rcp = big.tile([BS, 1], F32, tag="rcp")
        nc.vector.reciprocal(rcp, sum_e)
        w_bf = big.tile([16, L], BF16, tag="wbf")
        nc.vector.tensor_scalar(w_bf[0:BS, :], score_sb, rcp, None, op0=ALU.mult)
        # in-place again: score_sb becomes w = e / sum
        nc.vector.tensor_scalar(score_sb, score_sb, rcp, None, op0=ALU.mult)
        nc.sync.dma_start(out=w_out[:], in_=score_sb)

        # ---------- z = w^T @ c per batch ----------
        tc.strict_bb_all_engine_barrier()
        wT = big.tile([128, NLT, 16], BF16, tag="wT")
        nc.sync.dma_start(out=wT, in_=w_bf, transpose=True)

        for b in range(BS):
            zp = php.tile([1, D], F32, tag="ph")
            for lt in range(NLT):
                nc.tensor.matmul(
                    zp,
                    wT[:, lt, b : b + 1],
                    nat[b][:, lt, :],
                    start=(lt == 0),
                    stop=(lt == NLT - 1),
                )
            z1 = mp.tile([1, D], F32, tag="z1")
            nc.vector.tensor_copy(z1, zp)
            nc.sync.dma_start(out=z_out[b : b + 1, :], in_=z1)

    nc.finalize()
    return nc


def _prep_inputs(query, context, context_mask, Wq, Wc, Ws):
    ctx_bf = np.ascontiguousarray(context.astype(BF))               # [B,L,D]
    ctxT_bf = np.ascontiguousarray(ctx_bf.transpose(0, 2, 1))       # [B,D,L]
    ws2 = np.ascontiguousarray(
        np.asarray(Ws, np.float32)[:, 0].reshape(2, 128).T          # [128,2]
    )
    wq = np.ascontiguousarray(np.asarray(Wq, np.float32))
    wc = np.ascontiguousarray(np.asarray(Wc, np.float32))
    in_maps = []
    for c in range(NCORES):
        s = slice(c * BS, (c + 1) * BS)
        in_maps.append(
            {
                "ctx": ctx_bf[s],
                "ctxT": ctxT_bf[s],
                "qT": np.ascontiguousarray(
                    np.asarray(query, np.float32)[s, 0, :].T        # [D,BS]
                ),
                "mask": np.ascontiguousarray(np.asarray(context_mask, np.int32)[s]),
                "Wq": wq,
                "Wc": wc,
                "Ws2": ws2,
            }
        )
    return in_maps


def run(query, context, context_mask, Wq, Wc, Ws, trace=False):
    if "nc" not in _CACHE:
        _CACHE["nc"] = build()
    nc = _CACHE["nc"]
    in_maps = _prep_inputs(query, context, context_mask, Wq, Wc, Ws)
    res = run_bass_kernel_spmd(nc, in_maps, core_ids=list(range(NCORES)), trace=trace)
    w_full = np.concatenate(
        [np.asarray(res.results[i]["w_out"], np.float32) for i in range(NCORES)], axis=0
    )
    z_full = np.concatenate(
        [np.asarray(res.results[i]["z_out"], np.float32) for i in range(NCORES)], axis=0
    )
    return (w_full[..., None], z_full), res


def kernel(query, context, context_mask, Wq, Wc, Ws):
    (w, z), _ = run(query, context, context_mask, Wq, Wc, Ws, trace=False)
    return (w, z)


# revision 7
# speedup vs baseline: 1.1586x; 1.1586x over previous
"""Additive (Bahdanau) attention on 8 TRN2 NeuronCores, pure data parallel.

reference:
    h = tanh(q @ Wq + c @ Wc)         # [B,L,D]
    score = squeeze(h @ Ws)           # [B,L]
    score = where(mask==1, score, -1e25)
    weight = softmax(score)[..., None]
    z = sum(weight * c, -2)           # [B,D]
    returns (weight, z)

Sharding: batch B=32 split 4-per-core across 8 cores; weights replicated.

Device layout strategy (per core, batch shard size 4):
  - context arrives twice from host as bf16: natural [4,L,D] (for the z
    matmuls, contraction over L on partitions) and pre-transposed [4,D,L]
    (for the c@Wc matmuls, contraction over D on partitions).  This avoids
    any on-chip fp32 transpose (xbar DMA transpose is 16-bit only).
  - cW^T tiles accumulate in PSUM; ScalarE applies tanh with the per-
    partition bias qW^T fused; a second PE pass contracts with Ws into a
    batched [4, L] score in PSUM (per-batch column trick in the stationary).
  - masked softmax on the [4, L] rows (DVE+ACT, fused accum for the sum).
  - softmax weights are transposed via one 16-bit xbar DMA transpose and
    used as [128,1] stationaries for the z accumulation over L tiles.
"""
import sys

sys.path.insert(0, "/opt/trn_rl_repo")

import numpy as np
import ml_dtypes
from contextlib import ExitStack

import concourse.bass as bass
import concourse.tile as tile
from concourse import bacc, mybir
from concourse.bass_utils import run_bass_kernel_spmd

F32 = mybir.dt.float32
BF16 = mybir.dt.bfloat16
I32 = mybir.dt.int32
BF = ml_dtypes.bfloat16
AF = mybir.ActivationFunctionType
ALU = mybir.AluOpType

B, L, D = 32, 4096, 256
NCORES = 8
BS = B // NCORES          # 4 batches per core
LC = 1024                 # L-chunk for the matmul/tanh pipeline
NLC = L // LC             # 4
NLT = L // 128            # 32 l-tiles of 128
MASK_FILL = -1e25

_CACHE = {}


def build():
    nc = bacc.Bacc()
    ctx_p = nc.declare_dram_parameter("ctx", [BS, L, D], BF16, isOutput=False)
    ctxT_p = nc.declare_dram_parameter("ctxT", [BS, D, L], BF16, isOutput=False)
    qT_p = nc.declare_dram_parameter("qT", [D, BS], F32, isOutput=False)
    mask_p = nc.declare_dram_parameter("mask", [BS, L], I32, isOutput=False)
    wq_p = nc.declare_dram_parameter("Wq", [D, D], F32, isOutput=False)
    wc_p = nc.declare_dram_parameter("Wc", [D, D], F32, isOutput=False)
    ws_p = nc.declare_dram_parameter("Ws2", [128, 2], F32, isOutput=False)
    w_out = nc.declare_dram_parameter("w_out", [BS, L], F32, isOutput=True)
    z_out = nc.declare_dram_parameter("z_out", [BS, D], F32, isOutput=True)

    with ExitStack() as ctx:
        tc = ctx.enter_context(tile.TileContext(nc))
        big = ctx.enter_context(tc.tile_pool(name="big", bufs=1))
        hp = ctx.enter_context(tc.tile_pool(name="hp", bufs=4))
        mp = ctx.enter_context(tc.tile_pool(name="mp", bufs=2))
        php = ctx.enter_context(tc.tile_pool(name="php", bufs=3, space="PSUM"))
        psc = ctx.enter_context(tc.tile_pool(name="psc", bufs=1, space="PSUM"))

        # ---------- PE warmup: dense dummy matmuls while the first loads land
        warm = big.tile([128, 512], BF16, tag="warm")
        nc.vector.memset(warm, 0.0)
        for i in range(16):
            wps = psc.tile([BS, LC], F32, tag="sc")
            nc.tensor.matmul(
                wps[:, 0:512], warm[:, 0:BS], warm, start=True, stop=True
            )

        # ---------- resident loads (small first, z-only nat last) ----------
        qT_sb = big.tile([128, 2, BS], F32, tag="qT")
        nc.sync.dma_start(out=qT_sb, in_=qT_p[:].rearrange("(h p) b -> p h b", p=128))
        wq_sb = big.tile([128, 2, D], F32, tag="wq")
        nc.sync.dma_start(out=wq_sb, in_=wq_p[:].rearrange("(h p) n -> p h n", p=128))
        wc_sb = big.tile([128, 2, D], F32, tag="wc")
        nc.sync.dma_start(out=wc_sb, in_=wc_p[:].rearrange("(h p) n -> p h n", p=128))
        ws_sb = big.tile([128, 2], F32, tag="ws")
        nc.sync.dma_start(out=ws_sb, in_=ws_p[:])

        cT = []
        nat = []
        for b in range(BS):
            t = big.tile([128, 2, L], BF16, tag=f"cT{b}")
            nc.sync.dma_start(out=t, in_=ctxT_p[b].rearrange("(h p) l -> p h l", p=128))
            cT.append(t)
        for b in range(BS):
            t = big.tile([128, NLT, D], BF16, tag=f"nat{b}")
            nc.sync.dma_start(out=t, in_=ctx_p[b].rearrange("(lt p) d -> p lt d", p=128))
            nat.append(t)

        wc_bf = big.tile([128, 2, D], BF16, tag="wcbf")
        nc.vector.tensor_copy(wc_bf, wc_sb)
        ws_bf = big.tile([128, 2], BF16, tag="wsbf")
        nc.vector.tensor_copy(ws_bf, ws_sb)

        # stationary for the batched score matmul: col b = Ws half, rest 0
        wscol = big.tile([128, 2, BS, BS], BF16, tag="wscol")
        nc.vector.memset(wscol, 0.0)
        for mh in range(2):
            for b in range(BS):
                nc.vector.tensor_copy(wscol[:, mh, b, b : b + 1], ws_bf[:, mh : mh + 1])

        # ---------- qW^T = (q @ Wq)^T as per-partition bias [128, mh, b] ----------
        qwt = big.tile([128, 2, BS], F32, tag="qwt")
        for mh in range(2):
            pq = php.tile([128, BS], F32, tag="ph")
            for kh in range(2):
                nc.tensor.matmul(
                    pq,
                    wq_sb[:, kh, mh * 128 : (mh + 1) * 128],
                    qT_sb[:, kh, :],
                    start=(kh == 0),
                    stop=(kh == 1),
                )
            nc.vector.tensor_copy(qwt[:, mh, :], pq)

        # ---------- main loop: cW^T -> tanh -> score ----------
        score_sb = big.tile([BS, L], F32, tag="score")
        for lc in range(NLC):
            ls = lc * LC
            sc_ps = psc.tile([BS, LC], F32, tag="sc")
            first = True
            for b in range(BS):
                for mh in range(2):
                    ph = php.tile([128, LC], F32, tag="ph")
                    for kh in range(2):
                        for nh in range(LC // 512):
                            nc.tensor.matmul(
                                ph[:, nh * 512 : (nh + 1) * 512],
                                wc_bf[:, kh, mh * 128 : (mh + 1) * 128],
                                cT[b][:, kh, ls + nh * 512 : ls + (nh + 1) * 512],
                                start=(kh == 0),
                                stop=(kh == 1),
                            )
                    hb = hp.tile([128, LC], BF16, tag="h")
                    nc.scalar.activation(hb, ph, AF.Tanh, bias=qwt[:, mh, b : b + 1])
                    for nh in range(LC // 512):
                        nc.tensor.matmul(
                            sc_ps[:, nh * 512 : (nh + 1) * 512],
                            wscol[:, mh, b, :],
                            hb[:, nh * 512 : (nh + 1) * 512],
                            start=first,
                            stop=(b == BS - 1 and mh == 1),
                            skip_group_check=True,
                        )
                    first = False
            # apply additive mask while copying PSUM -> SBUF
            mstage = mp.tile([BS, LC], I32, tag="mstage")
            nc.sync.dma_start(out=mstage, in_=mask_p[:, ls : ls + LC])
            mt = mp.tile([BS, LC], F32, tag="mt")
            nc.vector.tensor_scalar(mt, mstage, 1e25, -1e25, op0=ALU.mult, op1=ALU.add)
            nc.vector.tensor_add(score_sb[:, ls : ls + LC], sc_ps, mt)

        # ---------- masked softmax over L, batched on partitions 0..3 ----------
        negmax = big.tile([BS, 1], F32, tag="negmax")
        nc.vector.tensor_reduce(
            negmax, score_sb, axis=mybir.AxisListType.X, op=ALU.max, negate=True
        )
        sum_e = big.tile([BS, 1], F32, tag="sume")
        # in-place: score_sb becomes e = exp(score - max); sum fused
        nc.scalar.activation(score_sb, score_sb, AF.Exp, bias=negmax, accum_out=sum_e)
        rcp = big.tile([BS, 1], F32, tag="rcp")
        nc.vector.reciprocal(rcp, sum_e)
        w_bf = big.tile([16, L], BF16, tag="wbf")
        nc.vector.tensor_scalar(w_bf[0:BS, :], score_sb, rcp, None, op0=ALU.mult)
        # in-place again: score_sb becomes w = e / sum
        nc.vector.tensor_scalar(score_sb, score_sb, rcp, None, op0=ALU.mult)
        nc.sync.dma_start(out=w_out[:], in_=score_sb)

        # ---------- z = w^T @ c per batch ----------
        tc.strict_bb_all_engine_barrier()
        wT = big.tile([128, NLT, 16], BF16, tag="wT")
        nc.sync.dma_start(out=wT, in_=w_bf, transpose=True)

        for b in range(BS):
            zp = php.tile([1, D], F32, tag="ph")
            for lt in range(NLT):
                nc.tensor.matmul(
                    zp,
                    wT[:, lt, b : b + 1],
                    nat[b][:, lt, :],
                    start=(lt == 0),
                    stop=(lt == NLT - 1),
                )
            z1 = mp.tile([1, D], F32, tag="z1")
            nc.vector.tensor_copy(z1, zp)
            nc.sync.dma_start(out=z_out[b : b + 1, :], in_=z1)

    nc.finalize()
    return nc


def _prep_inputs(query, context, context_mask, Wq, Wc, Ws):
    ctx_bf = np.ascontiguousarray(context.astype(BF))               # [B,L,D]
    ctxT_bf = np.ascontiguousarray(ctx_bf.transpose(0, 2, 1))       # [B,D,L]
    ws2 = np.ascontiguousarray(
        np.asarray(Ws, np.float32)[:, 0].reshape(2, 128).T          # [128,2]
    )
    wq = np.ascontiguousarray(np.asarray(Wq, np.float32))
    wc = np.ascontiguousarray(np.asarray(Wc, np.float32))
    in_maps = []
    for c in range(NCORES):
        s = slice(c * BS, (c + 1) * BS)
        in_maps.append(
            {
                "ctx": ctx_bf[s],
                "ctxT": ctxT_bf[s],
                "qT": np.ascontiguousarray(
                    np.asarray(query, np.float32)[s, 0, :].T        # [D,BS]
                ),
                "mask": np.ascontiguousarray(np.asarray(context_mask, np.int32)[s]),
                "Wq": wq,
                "Wc": wc,
                "Ws2": ws2,
            }
        )
    return in_maps


def run(query, context, context_mask, Wq, Wc, Ws, trace=False):
    if "nc" not in _CACHE:
        _CACHE["nc"] = build()
    nc = _CACHE["nc"]
    in_maps = _prep_inputs(query, context, context_mask, Wq, Wc, Ws)
    res = run_bass_kernel_spmd(nc, in_maps, core_ids=list(range(NCORES)), trace=trace)
    w_full = np.concatenate(
        [np.asarray(res.results[i]["w_out"], np.float32) for i in range(NCORES)], axis=0
    )
    z_full = np.concatenate(
        [np.asarray(res.results[i]["z_out"], np.float32) for i in range(NCORES)], axis=0
    )
    return (w_full[..., None], z_full), res


def kernel(query, context, context_mask, Wq, Wc, Ws):
    (w, z), _ = run(query, context, context_mask, Wq, Wc, Ws, trace=False)
    return (w, z)


# revision 8
# speedup vs baseline: 1.3482x; 1.1636x over previous
"""Additive (Bahdanau) attention on 8 TRN2 NeuronCores, pure data parallel.

reference:
    h = tanh(q @ Wq + c @ Wc)         # [B,L,D]
    score = squeeze(h @ Ws)           # [B,L]
    score = where(mask==1, score, -1e25)
    weight = softmax(score)[..., None]
    z = sum(weight * c, -2)           # [B,D]
    returns (weight, z)

Sharding: batch B=32 split 4-per-core across 8 cores; weights replicated.

Device layout strategy (per core, batch shard size 4):
  - context arrives twice from host as bf16: natural [4,L,D] (for the z
    matmuls, contraction over L on partitions) and pre-transposed [4,D,L]
    (for the c@Wc matmuls, contraction over D on partitions).  This avoids
    any on-chip fp32 transpose (xbar DMA transpose is 16-bit only).
  - cW^T tiles accumulate in PSUM; ScalarE applies tanh with the per-
    partition bias qW^T fused; a second PE pass contracts with Ws into a
    batched [4, L] score in PSUM (per-batch column trick in the stationary).
  - masked softmax on the [4, L] rows (DVE+ACT, fused accum for the sum).
  - softmax weights are transposed via one 16-bit xbar DMA transpose and
    used as [128,1] stationaries for the z accumulation over L tiles.
"""
import sys

sys.path.insert(0, "/opt/trn_rl_repo")

import numpy as np
import ml_dtypes
from contextlib import ExitStack

import concourse.bass as bass
import concourse.tile as tile
from concourse import bacc, mybir
from concourse.bass_utils import run_bass_kernel_spmd

F32 = mybir.dt.float32
BF16 = mybir.dt.bfloat16
I32 = mybir.dt.int32
BF = ml_dtypes.bfloat16
AF = mybir.ActivationFunctionType
ALU = mybir.AluOpType

B, L, D = 32, 4096, 256
NCORES = 8
BS = B // NCORES          # 4 batches per core
LC = 1024                 # L-chunk for the matmul/tanh pipeline
NLC = L // LC             # 4
NLT = L // 128            # 32 l-tiles of 128
MASK_FILL = -1e25

_CACHE = {}


def build():
    nc = bacc.Bacc()
    ctx_p = nc.declare_dram_parameter("ctx", [BS, L, D], BF16, isOutput=False)
    ctxT_p = nc.declare_dram_parameter("ctxT", [BS, D, L], BF16, isOutput=False)
    qT_p = nc.declare_dram_parameter("qT", [D, BS], F32, isOutput=False)
    mask_p = nc.declare_dram_parameter("mask", [BS, L], I32, isOutput=False)
    wq_p = nc.declare_dram_parameter("Wq", [D, D], F32, isOutput=False)
    wc_p = nc.declare_dram_parameter("Wc", [D, D], F32, isOutput=False)
    ws_p = nc.declare_dram_parameter("Ws2", [128, 2], F32, isOutput=False)
    w_out = nc.declare_dram_parameter("w_out", [BS, L], F32, isOutput=True)
    z_out = nc.declare_dram_parameter("z_out", [BS, D], F32, isOutput=True)

    with ExitStack() as ctx:
        tc = ctx.enter_context(tile.TileContext(nc))
        big = ctx.enter_context(tc.tile_pool(name="big", bufs=1))
        hp = ctx.enter_context(tc.tile_pool(name="hp", bufs=4))
        mp = ctx.enter_context(tc.tile_pool(name="mp", bufs=2))
        php = ctx.enter_context(tc.tile_pool(name="php", bufs=3, space="PSUM"))
        psc = ctx.enter_context(tc.tile_pool(name="psc", bufs=1, space="PSUM"))

        # ---------- PE warmup: dense dummy matmuls while the first loads land
        warm = big.tile([128, 512], BF16, tag="warm")
        nc.vector.memset(warm, 0.0)
        for i in range(16):
            wps = psc.tile([BS, LC], F32, tag="sc")
            nc.tensor.matmul(
                wps[:, 0:512], warm[:, 0:BS], warm, start=True, stop=True
            )

        # ---------- resident loads (small first, z-only nat last) ----------
        qT_sb = big.tile([128, 2, BS], F32, tag="qT")
        nc.sync.dma_start(out=qT_sb, in_=qT_p[:].rearrange("(h p) b -> p h b", p=128))
        wq_sb = big.tile([128, 2, D], F32, tag="wq")
        nc.sync.dma_start(out=wq_sb, in_=wq_p[:].rearrange("(h p) n -> p h n", p=128))
        wc_sb = big.tile([128, 2, D], F32, tag="wc")
        nc.sync.dma_start(out=wc_sb, in_=wc_p[:].rearrange("(h p) n -> p h n", p=128))
        ws_sb = big.tile([128, 2], F32, tag="ws")
        nc.sync.dma_start(out=ws_sb, in_=ws_p[:])

        cT = []
        nat = []
        for b in range(BS):
            t = big.tile([128, 2, L], BF16, tag=f"cT{b}")
            nc.sync.dma_start(out=t, in_=ctxT_p[b].rearrange("(h p) l -> p h l", p=128))
            cT.append(t)
        for b in range(BS):
            t = big.tile([128, NLT, D], BF16, tag=f"nat{b}")
            nc.sync.dma_start(out=t, in_=ctx_p[b].rearrange("(lt p) d -> p lt d", p=128))
            nat.append(t)

        wc_bf = big.tile([128, 2, D], BF16, tag="wcbf")
        nc.vector.tensor_copy(wc_bf, wc_sb)
        ws_bf = big.tile([128, 2], BF16, tag="wsbf")
        nc.vector.tensor_copy(ws_bf, ws_sb)

        # stationary for the batched score matmul: col b = Ws half, rest 0
        wscol = big.tile([128, 2, BS, BS], BF16, tag="wscol")
        nc.vector.memset(wscol, 0.0)
        for mh in range(2):
            for b in range(BS):
                nc.vector.tensor_copy(wscol[:, mh, b, b : b + 1], ws_bf[:, mh : mh + 1])

        # ---------- qW^T = (q @ Wq)^T as per-partition bias [128, mh, b] ----------
        qwt = big.tile([128, 2, BS], F32, tag="qwt")
        for mh in range(2):
            pq = php.tile([128, BS], F32, tag="ph")
            for kh in range(2):
                nc.tensor.matmul(
                    pq,
                    wq_sb[:, kh, mh * 128 : (mh + 1) * 128],
                    qT_sb[:, kh, :],
                    start=(kh == 0),
                    stop=(kh == 1),
                )
            nc.vector.tensor_copy(qwt[:, mh, :], pq)

        # ---------- main loop: cW^T -> tanh -> score ----------
        score_sb = big.tile([BS, L], F32, tag="score")
        for lc in range(NLC):
            ls = lc * LC
            sc_ps = psc.tile([BS, LC], F32, tag="sc")
            first = True
            for b in range(BS):
                for mh in range(2):
                    ph = php.tile([128, LC], F32, tag="ph")
                    for kh in range(2):
                        for nh in range(LC // 512):
                            nc.tensor.matmul(
                                ph[:, nh * 512 : (nh + 1) * 512],
                                wc_bf[:, kh, mh * 128 : (mh + 1) * 128],
                                cT[b][:, kh, ls + nh * 512 : ls + (nh + 1) * 512],
                                start=(kh == 0),
                                stop=(kh == 1),
                            )
                    hb = hp.tile([128, LC], BF16, tag="h")
                    nc.scalar.activation(hb, ph, AF.Tanh, bias=qwt[:, mh, b : b + 1])
                    for nh in range(LC // 512):
                        nc.tensor.matmul(
                            sc_ps[:, nh * 512 : (nh + 1) * 512],
                            wscol[:, mh, b, :],
                            hb[:, nh * 512 : (nh + 1) * 512],
                            start=first,
                            stop=(b == BS - 1 and mh == 1),
                            skip_group_check=True,
                        )
                    first = False
            # apply additive mask while copying PSUM -> SBUF
            mstage = mp.tile([BS, LC], I32, tag="mstage")
            nc.scalar.dma_start(out=mstage, in_=mask_p[:, ls : ls + LC])
            mt = mp.tile([BS, LC], F32, tag="mt")
            nc.vector.tensor_scalar(mt, mstage, 1e25, -1e25, op0=ALU.mult, op1=ALU.add)
            nc.vector.tensor_add(score_sb[:, ls : ls + LC], sc_ps, mt)

        # ---------- masked softmax over L, batched on partitions 0..3 ----------
        negmax = big.tile([BS, 1], F32, tag="negmax")
        nc.vector.tensor_reduce(
            negmax, score_sb, axis=mybir.AxisListType.X, op=ALU.max, negate=True
        )
        sum_e = big.tile([BS, 1], F32, tag="sume")
        # in-place: score_sb becomes e = exp(score - max); sum fused
        nc.scalar.activation(score_sb, score_sb, AF.Exp, bias=negmax, accum_out=sum_e)
        rcp = big.tile([BS, 1], F32, tag="rcp")
        nc.vector.reciprocal(rcp, sum_e)
        w_bf = big.tile([16, L], BF16, tag="wbf")
        nc.vector.tensor_scalar(w_bf[0:BS, :], score_sb, rcp, None, op0=ALU.mult)
        # in-place again: score_sb becomes w = e / sum
        nc.vector.tensor_scalar(score_sb, score_sb, rcp, None, op0=ALU.mult)
        nc.sync.dma_start(out=w_out[:], in_=score_sb)

        # ---------- z = w^T @ c per batch ----------
        tc.strict_bb_all_engine_barrier()
        wT = big.tile([128, NLT, 16], BF16, tag="wT")
        nc.sync.dma_start(out=wT, in_=w_bf, transpose=True)

        for b in range(BS):
            zp = php.tile([1, D], F32, tag="ph")
            for lt in range(NLT):
                nc.tensor.matmul(
                    zp,
                    wT[:, lt, b : b + 1],
                    nat[b][:, lt, :],
                    start=(lt == 0),
                    stop=(lt == NLT - 1),
                )
            z1 = mp.tile([1, D], F32, tag="z1")
            nc.vector.tensor_copy(z1, zp)
            nc.sync.dma_start(out=z_out[b : b + 1, :], in_=z1)

    nc.finalize()
    return nc


def _prep_inputs(query, context, context_mask, Wq, Wc, Ws):
    ctx_bf = np.ascontiguousarray(context.astype(BF))               # [B,L,D]
    ctxT_bf = np.ascontiguousarray(ctx_bf.transpose(0, 2, 1))       # [B,D,L]
    ws2 = np.ascontiguousarray(
        np.asarray(Ws, np.float32)[:, 0].reshape(2, 128).T          # [128,2]
    )
    wq = np.ascontiguousarray(np.asarray(Wq, np.float32))
    wc = np.ascontiguousarray(np.asarray(Wc, np.float32))
    in_maps = []
    for c in range(NCORES):
        s = slice(c * BS, (c + 1) * BS)
        in_maps.append(
            {
                "ctx": ctx_bf[s],
                "ctxT": ctxT_bf[s],
                "qT": np.ascontiguousarray(
                    np.asarray(query, np.float32)[s, 0, :].T        # [D,BS]
                ),
                "mask": np.ascontiguousarray(np.asarray(context_mask, np.int32)[s]),
                "Wq": wq,
                "Wc": wc,
                "Ws2": ws2,
            }
        )
    return in_maps


def run(query, context, context_mask, Wq, Wc, Ws, trace=False):
    if "nc" not in _CACHE:
        _CACHE["nc"] = build()
    nc = _CACHE["nc"]
    in_maps = _prep_inputs(query, context, context_mask, Wq, Wc, Ws)
    res = run_bass_kernel_spmd(nc, in_maps, core_ids=list(range(NCORES)), trace=trace)
    w_full = np.concatenate(
        [np.asarray(res.results[i]["w_out"], np.float32) for i in range(NCORES)], axis=0
    )
    z_full = np.concatenate(
        [np.asarray(res.results[i]["z_out"], np.float32) for i in range(NCORES)], axis=0
    )
    return (w_full[..., None], z_full), res


def kernel(query, context, context_mask, Wq, Wc, Ws):
    (w, z), _ = run(query, context, context_mask, Wq, Wc, Ws, trace=False)
    return (w, z)


# revision 11
# speedup vs baseline: 1.5145x; 1.1234x over previous
"""Additive (Bahdanau) attention on 8 TRN2 NeuronCores, pure data parallel.

reference:
    h = tanh(q @ Wq + c @ Wc)         # [B,L,D]
    score = squeeze(h @ Ws)           # [B,L]
    score = where(mask==1, score, -1e25)
    weight = softmax(score)[..., None]
    z = sum(weight * c, -2)           # [B,D]
    returns (weight, z)

Sharding: batch B=32 split 4-per-core across 8 cores; weights replicated.

Device layout strategy (per core, batch shard size 4):
  - context arrives twice from host as bf16: natural [4,L,D] (for the z
    matmuls, contraction over L on partitions) and pre-transposed [4,D,L]
    (for the c@Wc matmuls, contraction over D on partitions).  This avoids
    any on-chip fp32 transpose (xbar DMA transpose is 16-bit only).
  - cW^T tiles accumulate in PSUM; ScalarE applies tanh with the per-
    partition bias qW^T fused; a second PE pass contracts with Ws into a
    batched [4, L] score in PSUM (per-batch column trick in the stationary).
  - masked softmax on the [4, L] rows (DVE+ACT, fused accum for the sum).
  - softmax weights are transposed via one 16-bit xbar DMA transpose and
    used as [128,1] stationaries for the z accumulation over L tiles.
"""
import sys

sys.path.insert(0, "/opt/trn_rl_repo")

import numpy as np
import ml_dtypes
from contextlib import ExitStack

import concourse.bass as bass
import concourse.tile as tile
from concourse import bacc, mybir
from concourse.bass_utils import run_bass_kernel_spmd

F32 = mybir.dt.float32
BF16 = mybir.dt.bfloat16
I32 = mybir.dt.int32
BF = ml_dtypes.bfloat16
AF = mybir.ActivationFunctionType
ALU = mybir.AluOpType

B, L, D = 32, 4096, 256
NCORES = 8
BS = B // NCORES          # 4 batches per core
LC = 1024                 # L-chunk for the matmul/tanh pipeline
NLC = L // LC             # 4
NLT = L // 128            # 32 l-tiles of 128
MASK_FILL = -1e25

_CACHE = {}


def build():
    nc = bacc.Bacc()
    ctx_p = nc.declare_dram_parameter("ctx", [BS, L, D], BF16, isOutput=False)
    ctxT_p = nc.declare_dram_parameter("ctxT", [BS, D, L], BF16, isOutput=False)
    qT_p = nc.declare_dram_parameter("qT", [D, BS], F32, isOutput=False)
    mask_p = nc.declare_dram_parameter("mask", [BS, L], I32, isOutput=False)
    wq_p = nc.declare_dram_parameter("Wq", [D, D], F32, isOutput=False)
    wc_p = nc.declare_dram_parameter("Wc", [D, D], F32, isOutput=False)
    ws_p = nc.declare_dram_parameter("Ws2", [128, 2], F32, isOutput=False)
    w_out = nc.declare_dram_parameter("w_out", [BS, L], F32, isOutput=True)
    z_out = nc.declare_dram_parameter("z_out", [BS, D], F32, isOutput=True)

    with ExitStack() as ctx:
        tc = ctx.enter_context(tile.TileContext(nc))
        big = ctx.enter_context(tc.tile_pool(name="big", bufs=1))
        hp = ctx.enter_context(tc.tile_pool(name="hp", bufs=4))
        mp = ctx.enter_context(tc.tile_pool(name="mp", bufs=2))
        php = ctx.enter_context(tc.tile_pool(name="php", bufs=3, space="PSUM"))
        psc = ctx.enter_context(tc.tile_pool(name="psc", bufs=1, space="PSUM"))

        # ---------- PE warmup: dense dummy matmuls while the first loads land
        warm = big.tile([128, 512], BF16, tag="warm")
        nc.vector.memset(warm, 0.0)
        for i in range(16):
            wps = psc.tile([BS, LC], F32, tag="sc")
            nc.tensor.matmul(
                wps[:, 0:512], warm[:, 0:BS], warm, start=True, stop=True
            )

        # ---------- resident loads (small first, z-only nat last) ----------
        qT_sb = big.tile([128, 2, BS], F32, tag="qT")
        nc.sync.dma_start(out=qT_sb, in_=qT_p[:].rearrange("(h p) b -> p h b", p=128))
        wq_sb = big.tile([128, 2, D], F32, tag="wq")
        nc.sync.dma_start(out=wq_sb, in_=wq_p[:].rearrange("(h p) n -> p h n", p=128))
        wc_sb = big.tile([128, 2, D], F32, tag="wc")
        nc.sync.dma_start(out=wc_sb, in_=wc_p[:].rearrange("(h p) n -> p h n", p=128))
        ws_sb = big.tile([128, 2], F32, tag="ws")
        nc.sync.dma_start(out=ws_sb, in_=ws_p[:])

        # chunked so compute on (lc, b) can start as soon as its 512KB lands
        cT = [[None] * NLC for _ in range(BS)]
        for lc in range(NLC):
            for b in range(BS):
                t = big.tile([128, 2, LC], BF16, tag=f"cT{b}_{lc}")
                nc.sync.dma_start(
                    out=t,
                    in_=ctxT_p[b][:, lc * LC : (lc + 1) * LC].rearrange(
                        "(h p) l -> p h l", p=128
                    ),
                )
                cT[b][lc] = t
        nat = []
        for b in range(BS):
            t = big.tile([128, NLT, D], BF16, tag=f"nat{b}")
            nc.sync.dma_start(out=t, in_=ctx_p[b].rearrange("(lt p) d -> p lt d", p=128))
            nat.append(t)

        wc_bf = big.tile([128, 2, D], BF16, tag="wcbf")
        nc.vector.tensor_copy(wc_bf, wc_sb)
        ws_bf = big.tile([128, 2], BF16, tag="wsbf")
        nc.vector.tensor_copy(ws_bf, ws_sb)

        # stationary for the batched score matmul: col b = Ws half, rest 0
        wscol = big.tile([128, 2, BS, BS], BF16, tag="wscol")
        nc.vector.memset(wscol, 0.0)
        for mh in range(2):
            for b in range(BS):
                nc.vector.tensor_copy(wscol[:, mh, b, b : b + 1], ws_bf[:, mh : mh + 1])

        # ---------- qW^T = (q @ Wq)^T as per-partition bias [128, mh, b] ----------
        qwt = big.tile([128, 2, BS], F32, tag="qwt")
        for mh in range(2):
            pq = php.tile([128, BS], F32, tag="ph")
            for kh in range(2):
                nc.tensor.matmul(
                    pq,
                    wq_sb[:, kh, mh * 128 : (mh + 1) * 128],
                    qT_sb[:, kh, :],
                    start=(kh == 0),
                    stop=(kh == 1),
                )
            nc.vector.tensor_copy(qwt[:, mh, :], pq)

        # ---------- main loop: cW^T -> tanh -> score ----------
        score_sb = big.tile([BS, L], F32, tag="score")
        mx4 = big.tile([BS, NLC], F32, tag="mx4")
        for lc in range(NLC):
            ls = lc * LC
            sc_ps = psc.tile([BS, LC], F32, tag="sc")
            first = True
            for b in range(BS):
                for mh in range(2):
                    ph = php.tile([128, LC], F32, tag="ph")
                    for kh in range(2):
                        for nh in range(LC // 512):
                            nc.tensor.matmul(
                                ph[:, nh * 512 : (nh + 1) * 512],
                                wc_bf[:, kh, mh * 128 : (mh + 1) * 128],
                                cT[b][lc][:, kh, nh * 512 : (nh + 1) * 512],
                                start=(kh == 0),
                                stop=(kh == 1),
                            )
                    hb = hp.tile([128, LC], BF16, tag="h")
                    nc.scalar.activation(hb, ph, AF.Tanh, bias=qwt[:, mh, b : b + 1])
                    for nh in range(LC // 512):
                        nc.tensor.matmul(
                            sc_ps[:, nh * 512 : (nh + 1) * 512],
                            wscol[:, mh, b, :],
                            hb[:, nh * 512 : (nh + 1) * 512],
                            start=first,
                            stop=(b == BS - 1 and mh == 1),
                            skip_group_check=True,
                        )
                    first = False
            # apply additive mask while copying PSUM -> SBUF
            mstage = mp.tile([BS, LC], I32, tag="mstage")
            nc.scalar.dma_start(out=mstage, in_=mask_p[:, ls : ls + LC])
            mt = mp.tile([BS, LC], F32, tag="mt")
            nc.vector.tensor_scalar(mt, mstage, 1e25, -1e25, op0=ALU.mult, op1=ALU.add)
            nc.vector.tensor_add(score_sb[:, ls : ls + LC], sc_ps, mt)
            nc.vector.tensor_reduce(
                mx4[:, lc : lc + 1],
                score_sb[:, ls : ls + LC],
                axis=mybir.AxisListType.X,
                op=ALU.max,
            )

        # keep the PE warm through the softmax bubble so z runs at 2.4 GHz
        for i in range(24):
            wps = psc.tile([BS, LC], F32, tag="sc")
            nc.tensor.matmul(wps[:, 0:512], warm[:, 0:BS], warm, start=True, stop=True)

        # ---------- masked softmax over L, batched on partitions 0..3 ----------
        negmax = big.tile([BS, 1], F32, tag="negmax")
        nc.vector.tensor_reduce(
            negmax, mx4, axis=mybir.AxisListType.X, op=ALU.max, negate=True
        )
        sum_e = big.tile([BS, 1], F32, tag="sume")
        # in-place: score_sb becomes e = exp(score - max); sum fused
        nc.scalar.activation(score_sb, score_sb, AF.Exp, bias=negmax, accum_out=sum_e)
        rcp = big.tile([BS, 1], F32, tag="rcp")
        nc.vector.reciprocal(rcp, sum_e)
        w_bf = big.tile([16, L], BF16, tag="wbf")
        nc.vector.tensor_scalar(w_bf[0:BS, :], score_sb, rcp, None, op0=ALU.mult)
        # f32 weight output via SWDGE cast-DMA straight from the bf16 tile
        nc.gpsimd.dma_start(out=w_out[:], in_=w_bf[0:BS, :])

        # ---------- z = w^T @ c per batch ----------
        tc.strict_bb_all_engine_barrier()
        wT = big.tile([128, NLT, 16], BF16, tag="wT")
        nc.sync.dma_start(out=wT, in_=w_bf, transpose=True)

        for b in range(BS):
            zp = php.tile([1, D], F32, tag="ph")
            for lt in range(NLT):
                nc.tensor.matmul(
                    zp,
                    wT[:, lt, b : b + 1],
                    nat[b][:, lt, :],
                    start=(lt == 0),
                    stop=(lt == NLT - 1),
                )
            z1 = mp.tile([1, D], F32, tag="z1")
            nc.vector.tensor_copy(z1, zp)
            nc.sync.dma_start(out=z_out[b : b + 1, :], in_=z1)

    nc.finalize()
    return nc


def _prep_inputs(query, context, context_mask, Wq, Wc, Ws):
    ctx_bf = np.ascontiguousarray(context.astype(BF))               # [B,L,D]
    ctxT_bf = np.ascontiguousarray(ctx_bf.transpose(0, 2, 1))       # [B,D,L]
    ws2 = np.ascontiguousarray(
        np.asarray(Ws, np.float32)[:, 0].reshape(2, 128).T          # [128,2]
    )
    wq = np.ascontiguousarray(np.asarray(Wq, np.float32))
    wc = np.ascontiguousarray(np.asarray(Wc, np.float32))
    in_maps = []
    for c in range(NCORES):
        s = slice(c * BS, (c + 1) * BS)
        in_maps.append(
            {
                "ctx": ctx_bf[s],
                "ctxT": ctxT_bf[s],
                "qT": np.ascontiguousarray(
                    np.asarray(query, np.float32)[s, 0, :].T        # [D,BS]
                ),
                "mask": np.ascontiguousarray(np.asarray(context_mask, np.int32)[s]),
                "Wq": wq,
                "Wc": wc,
                "Ws2": ws2,
            }
        )
    return in_maps


def run(query, context, context_mask, Wq, Wc, Ws, trace=False):
    if "nc" not in _CACHE:
        _CACHE["nc"] = build()
    nc = _CACHE["nc"]
    in_maps = _prep_inputs(query, context, context_mask, Wq, Wc, Ws)
    res = run_bass_kernel_spmd(nc, in_maps, core_ids=list(range(NCORES)), trace=trace)
    w_full = np.concatenate(
        [np.asarray(res.results[i]["w_out"], np.float32) for i in range(NCORES)], axis=0
    )
    z_full = np.concatenate(
        [np.asarray(res.results[i]["z_out"], np.float32) for i in range(NCORES)], axis=0
    )
    return (w_full[..., None], z_full), res


def kernel(query, context, context_mask, Wq, Wc, Ws):
    (w, z), _ = run(query, context, context_mask, Wq, Wc, Ws, trace=False)
    return (w, z)


# revision 16
# speedup vs baseline: 1.6389x; 1.0821x over previous
"""Additive (Bahdanau) attention on 8 TRN2 NeuronCores, pure data parallel.

reference:
    h = tanh(q @ Wq + c @ Wc)         # [B,L,D]
    score = squeeze(h @ Ws)           # [B,L]
    score = where(mask==1, score, -1e25)
    weight = softmax(score)[..., None]
    z = sum(weight * c, -2)           # [B,D]
    returns (weight, z)

Sharding: batch B=32 split 4-per-core across 8 cores; weights replicated.

Device layout strategy (per core, batch shard size 4):
  - context arrives twice from host as bf16: natural [4,L,D] (for the z
    matmuls, contraction over L on partitions) and pre-transposed [4,D,L]
    (for the c@Wc matmuls, contraction over D on partitions).  This avoids
    any on-chip fp32 transpose (xbar DMA transpose is 16-bit only).
  - cW^T tiles accumulate in PSUM; ScalarE applies tanh with the per-
    partition bias qW^T fused; a second PE pass contracts with Ws into a
    batched [4, L] score in PSUM (per-batch column trick in the stationary).
  - masked softmax on the [4, L] rows (DVE+ACT, fused accum for the sum).
  - softmax weights are transposed via one 16-bit xbar DMA transpose and
    used as [128,1] stationaries for the z accumulation over L tiles.
"""
import sys

sys.path.insert(0, "/opt/trn_rl_repo")

import numpy as np
import ml_dtypes
from contextlib import ExitStack

import concourse.bass as bass
import concourse.tile as tile
from concourse import bacc, mybir
from concourse import bass_utils as _bu
from concourse.bass_utils import run_bass_kernel_spmd



F32 = mybir.dt.float32
BF16 = mybir.dt.bfloat16
I32 = mybir.dt.int32
BF = ml_dtypes.bfloat16
AF = mybir.ActivationFunctionType
ALU = mybir.AluOpType

B, L, D = 32, 4096, 256
NCORES = 8
BS = B // NCORES          # 4 batches per core
LC = 1024                 # L-chunk for the matmul/tanh pipeline
NLC = L // LC             # 4
NLT = L // 128            # 32 l-tiles of 128
MASK_FILL = -1e25

_CACHE = {}


def build():
    nc = bacc.Bacc()
    ctx_p = nc.declare_dram_parameter("ctx", [BS, L, D], BF16, isOutput=False)
    ctxT_p = nc.declare_dram_parameter("ctxT", [BS, D, L], BF16, isOutput=False)
    qT_p = nc.declare_dram_parameter("qT", [D, BS], F32, isOutput=False)
    mask_p = nc.declare_dram_parameter("mask", [BS, L], I32, isOutput=False)
    wq_p = nc.declare_dram_parameter("Wq", [D, D], F32, isOutput=False)
    wc_p = nc.declare_dram_parameter("Wc", [D, D], F32, isOutput=False)
    ws_p = nc.declare_dram_parameter("Ws2", [128, 2], F32, isOutput=False)
    w_out = nc.declare_dram_parameter("w_out", [BS, L], F32, isOutput=True)
    z_out = nc.declare_dram_parameter("z_out", [BS, D], F32, isOutput=True)

    with ExitStack() as ctx:
        tc = ctx.enter_context(tile.TileContext(nc))
        big = ctx.enter_context(tc.tile_pool(name="big", bufs=1))
        hp = ctx.enter_context(tc.tile_pool(name="hp", bufs=4))
        mp = ctx.enter_context(tc.tile_pool(name="mp", bufs=2))
        php = ctx.enter_context(tc.tile_pool(name="php", bufs=3, space="PSUM"))
        psc = ctx.enter_context(tc.tile_pool(name="psc", bufs=1, space="PSUM"))

        # ---------- PE warmup: dense dummy matmuls while the first loads land
        warm = big.tile([128, 512], BF16, tag="warm")
        nc.vector.memset(warm, 0.0)
        wps0 = psc.tile([BS, LC], F32, tag="sc")
        for i in range(20):
            nc.tensor.matmul(
                wps0[:, 0:512], warm[:, 0:BS], warm, start=True, stop=True
            )

        # ---------- resident loads (small first, z-only nat last) ----------
        qT_sb = big.tile([128, 2, BS], F32, tag="qT")
        nc.sync.dma_start(out=qT_sb, in_=qT_p[:].rearrange("(h p) b -> p h b", p=128))
        wq_sb = big.tile([128, 2, D], F32, tag="wq")
        nc.sync.dma_start(out=wq_sb, in_=wq_p[:].rearrange("(h p) n -> p h n", p=128))
        wc_sb = big.tile([128, 2, D], F32, tag="wc")
        nc.sync.dma_start(out=wc_sb, in_=wc_p[:].rearrange("(h p) n -> p h n", p=128))
        ws_sb = big.tile([128, 2], F32, tag="ws")
        nc.sync.dma_start(out=ws_sb, in_=ws_p[:])

        # chunked so compute on (lc, b) can start as soon as its 512KB lands
        cT = [[None] * NLC for _ in range(BS)]
        for lc in range(NLC):
            for b in range(BS):
                t = big.tile([128, 2, LC], BF16, tag=f"cT{b}_{lc}")
                nc.sync.dma_start(
                    out=t,
                    in_=ctxT_p[b][:, lc * LC : (lc + 1) * LC].rearrange(
                        "(h p) l -> p h l", p=128
                    ),
                )
                cT[b][lc] = t
        nat = []
        for b in range(BS):
            t = big.tile([128, NLT, D], BF16, tag=f"nat{b}")
            nc.sync.dma_start(out=t, in_=ctx_p[b].rearrange("(lt p) d -> p lt d", p=128))
            nat.append(t)

        wc_bf = big.tile([128, 2, D], BF16, tag="wcbf")
        nc.vector.tensor_copy(wc_bf, wc_sb)
        ws_bf = big.tile([128, 2], BF16, tag="wsbf")
        nc.vector.tensor_copy(ws_bf, ws_sb)

        # stationary for the batched score matmul: col b = Ws half, rest 0
        wscol = big.tile([128, 2, BS, BS], BF16, tag="wscol")
        nc.vector.memset(wscol, 0.0)
        for mh in range(2):
            for b in range(BS):
                nc.vector.tensor_copy(wscol[:, mh, b, b : b + 1], ws_bf[:, mh : mh + 1])

        # ---------- qW^T = (q @ Wq)^T as per-partition bias [128, mh, b] ----------
        qwt = big.tile([128, 2, BS], F32, tag="qwt")
        for mh in range(2):
            pq = php.tile([128, BS], F32, tag="ph")
            for kh in range(2):
                nc.tensor.matmul(
                    pq,
                    wq_sb[:, kh, mh * 128 : (mh + 1) * 128],
                    qT_sb[:, kh, :],
                    start=(kh == 0),
                    stop=(kh == 1),
                )
            nc.vector.tensor_copy(qwt[:, mh, :], pq)

        # ---------- main loop: cW^T -> tanh -> score ----------
        score_sb = big.tile([BS, L], F32, tag="score")
        mx4 = big.tile([BS, NLC], F32, tag="mx4")
        for lc in range(NLC):
            ls = lc * LC
            sc_ps = psc.tile([BS, LC], F32, tag="sc")
            first = True
            for b in range(BS):
                for mh in range(2):
                    ph = php.tile([128, LC], F32, tag="ph")
                    for kh in range(2):
                        for nh in range(LC // 512):
                            nc.tensor.matmul(
                                ph[:, nh * 512 : (nh + 1) * 512],
                                wc_bf[:, kh, mh * 128 : (mh + 1) * 128],
                                cT[b][lc][:, kh, nh * 512 : (nh + 1) * 512],
                                start=(kh == 0),
                                stop=(kh == 1),
                            )
                    hb = hp.tile([128, LC], BF16, tag="h")
                    nc.scalar.activation(hb, ph, AF.Tanh, bias=qwt[:, mh, b : b + 1])
                    for nh in range(LC // 512):
                        nc.tensor.matmul(
                            sc_ps[:, nh * 512 : (nh + 1) * 512],
                            wscol[:, mh, b, :],
                            hb[:, nh * 512 : (nh + 1) * 512],
                            start=first,
                            stop=(b == BS - 1 and mh == 1),
                            skip_group_check=True,
                        )
                    first = False
            # apply additive mask while copying PSUM -> SBUF
            mstage = mp.tile([BS, LC], I32, tag="mstage")
            nc.scalar.dma_start(out=mstage, in_=mask_p[:, ls : ls + LC])
            mt = mp.tile([BS, LC], F32, tag="mt")
            nc.vector.tensor_scalar(mt, mstage, 1e25, -1e25, op0=ALU.mult, op1=ALU.add)
            nc.vector.tensor_add(score_sb[:, ls : ls + LC], sc_ps, mt)
            nc.vector.tensor_reduce(
                mx4[:, lc : lc + 1],
                score_sb[:, ls : ls + LC],
                axis=mybir.AxisListType.X,
                op=ALU.max,
            )

        # keep the PE warm through the softmax bubble so z runs at 2.4 GHz
        wps1 = psc.tile([BS, LC], F32, tag="sc")
        for i in range(40):
            nc.tensor.matmul(wps1[:, 0:512], warm[:, 0:BS], warm, start=True, stop=True)

        # ---------- masked softmax over L, batched on partitions 0..3 ----------
        negmax = big.tile([BS, 1], F32, tag="negmax")
        nc.vector.tensor_reduce(
            negmax, mx4, axis=mybir.AxisListType.X, op=ALU.max, negate=True
        )
        sum_e = big.tile([BS, 1], F32, tag="sume")
        # in-place: score_sb becomes e = exp(score - max); sum fused
        nc.scalar.activation(score_sb, score_sb, AF.Exp, bias=negmax, accum_out=sum_e)
        rcp = big.tile([BS, 1], F32, tag="rcp")
        nc.vector.reciprocal(rcp, sum_e)
        w_bf = big.tile([16, L], BF16, tag="wbf")
        nc.vector.tensor_scalar(w_bf[0:BS, :], score_sb, rcp, None, op0=ALU.mult)
        # f32 weight output via SWDGE cast-DMA straight from the bf16 tile
        nc.gpsimd.dma_start(out=w_out[:], in_=w_bf[0:BS, :])

        # ---------- z = w^T @ c per batch ----------
        wT = big.tile([128, NLT, 16], BF16, tag="wT")
        nc.sync.dma_start(out=wT, in_=w_bf, transpose=True)

        for b in range(BS):
            zp = php.tile([1, D], F32, tag="ph")
            for lt in range(NLT):
                nc.tensor.matmul(
                    zp,
                    wT[:, lt, b : b + 1],
                    nat[b][:, lt, :],
                    start=(lt == 0),
                    stop=(lt == NLT - 1),
                )
            z1 = mp.tile([1, D], F32, tag="z1")
            nc.vector.tensor_copy(z1, zp)
            nc.sync.dma_start(out=z_out[b : b + 1, :], in_=z1)

    nc.finalize()
    return nc


def _prep_inputs(query, context, context_mask, Wq, Wc, Ws):
    ctx_bf = np.ascontiguousarray(context.astype(BF))               # [B,L,D]
    ctxT_bf = np.ascontiguousarray(ctx_bf.transpose(0, 2, 1))       # [B,D,L]
    ws2 = np.ascontiguousarray(
        np.asarray(Ws, np.float32)[:, 0].reshape(2, 128).T          # [128,2]
    )
    wq = np.ascontiguousarray(np.asarray(Wq, np.float32))
    wc = np.ascontiguousarray(np.asarray(Wc, np.float32))
    in_maps = []
    for c in range(NCORES):
        s = slice(c * BS, (c + 1) * BS)
        in_maps.append(
            {
                "ctx": ctx_bf[s],
                "ctxT": ctxT_bf[s],
                "qT": np.ascontiguousarray(
                    np.asarray(query, np.float32)[s, 0, :].T        # [D,BS]
                ),
                "mask": np.ascontiguousarray(np.asarray(context_mask, np.int32)[s]),
                "Wq": wq,
                "Wc": wc,
                "Ws2": ws2,
            }
        )
    return in_maps


def run(query, context, context_mask, Wq, Wc, Ws, trace=False):
    if "nc" not in _CACHE:
        _CACHE["nc"] = build()
    nc = _CACHE["nc"]
    in_maps = _prep_inputs(query, context, context_mask, Wq, Wc, Ws)
    res = run_bass_kernel_spmd(nc, in_maps, core_ids=list(range(NCORES)), trace=trace)
    w_full = np.concatenate(
        [np.asarray(res.results[i]["w_out"], np.float32) for i in range(NCORES)], axis=0
    )
    z_full = np.concatenate(
        [np.asarray(res.results[i]["z_out"], np.float32) for i in range(NCORES)], axis=0
    )
    return (w_full[..., None], z_full), res


def kernel(query, context, context_mask, Wq, Wc, Ws):
    (w, z), _ = run(query, context, context_mask, Wq, Wc, Ws, trace=False)
    return (w, z)
